# revision 5
# baseline (speedup 1.0000x reference)
"""AdMSoftmaxLoss fused distributed kernel for 8 TRN2 NeuronCores (v2).

Math (reference):
    xn = x / ||x||                     # row-L2-normalized embeddings
    wf = xn @ W.T                      # [N, C] logits
    tgt = wf[i, y_i]
    num = S * (tgt - M)
    excl = sum_c exp(S*wf) - exp(S*tgt)
    L = num - log(exp(num) + excl);  loss = -mean(L)

Strategy (v2): pure data-parallel over N (2048 rows/core), no collectives.
The scale S/||x_i|| is folded into x on the HOST (xs = S*x/||x||), so the
device matmul produces final logits directly and needs no per-row scale.
  - PE: fp8e4 DoubleRow matmuls (K=256 in one pass, 0.5 cyc/row): W is
    pre-scaled by 16 on host for fp8 range; the 1/16 is folded into the
    consumers.  PE stream ~35us -- far off the critical path.
  - The exp+row-sum work (20.5M elements/core) is SPLIT between the two
    engines that can read PSUM:
      * ScalarE (ACT): exp activation with scale=1/16 and accum_out
        (~2.28us per 2048-col chunk incl. the accumulator read), chunks
        {0,2,4} of each tile;
      * VectorE (DVE): Schraudolph bit-trick exp -- tensor_scalar affine
        fp32(PSUM) -> int16 bf16-bits (round-to-nearest), then one
        scalar_tensor_tensor that adds the two bf16 halves elementwise and
        accum-sums the result (~3.3us per chunk), chunks {1,3}.
    Schraudolph bf16 has ~+-4% sawtooth error, bias-corrected via the magic
    constant; per-row sums average it to ~0.3%, negligible vs the 2e-2 gate.
  - Target logits S*tgt come from DVE dot products of bf16 xs rows with the
    host-gathered g = W[labels] rows (accum_out).
  - Final log via the DVE exponent/poly bit-trick (no ACT table reload).
Per-row L values are DMA'd out; the host concatenates and means.

Measured: see test.py (baseline v1 was 189us HW; v2 targets ~120-130us).
"""

import numpy as np
import ml_dtypes

import concourse.mybir as mybir
import concourse.tile as tile
from concourse import bacc
from concourse.bass_utils import run_bass_kernel_spmd

N, D, C = 16384, 256, 10000
S, M = 30.0, 0.4
NCORES = 8
NS = N // NCORES      # 2048 rows per core
NT = NS // 128        # 16 n-tiles of 128 rows
KT = D // 128         # 2 k-slices (one DoubleRow pass)
CG = [2048, 2048, 2048, 2048, 1808]   # class-dim groups (sum = C)
NG = len(CG)
assert sum(CG) == C

_F32 = mybir.dt.float32
_BF16 = mybir.dt.bfloat16
_I16 = mybir.dt.int16
_I32 = mybir.dt.int32
_F8 = mybir.dt.float8e4

LN2 = float(np.log(2.0))
WSCALE = 16.0                       # host pre-scale on W for fp8 range
A16 = 128.0 / LN2 / WSCALE          # Schraudolph slope on 16x logits
B16 = 16256.0 - 7.37                # bf16 magic offset, mean-unbiased
SM = S * M

N_WARMUP_MM = 3


def _dve_gis(t):
    """Which class-groups of tile t the DVE (vs ACT) consumes."""
    return (2,) if t % 4 == 3 else (1, 3)


def _build_nc(ns=NS, cg=tuple(CG), c=C):
    nt = ns // 128
    cg = list(cg)
    nc = bacc.Bacc("TRN2", target_bir_lowering=False)
    AF = mybir.ActivationFunctionType
    NT, C = nt, c  # noqa: N806
    CG, NS = cg, ns  # noqa: N806
    NG = len(CG)  # noqa: N806
    NH = NT // 2  # noqa: N806
    DR = mybir.MatmulPerfMode.DoubleRow  # noqa: N806
    mult = mybir.AluOpType.mult
    sub = mybir.AluOpType.subtract
    addop = mybir.AluOpType.add

    xt_ext = nc.declare_dram_parameter("xt", [128, KT, NS], _F8, isOutput=False)
    wt_ext = nc.declare_dram_parameter("wt", [128, KT, C], _F8, isOutput=False)
    xf_ext = nc.declare_dram_parameter("xf", [128, NT, D], _BF16, isOutput=False)
    g_ext = nc.declare_dram_parameter("g", [128, NT, D], _BF16, isOutput=False)
    out_ext = nc.declare_dram_parameter("out", [128, NT], _F32, isOutput=True)

    with tile.TileContext(nc) as tc:
        with (
            tc.tile_pool(name="big", bufs=1) as big,
            tc.tile_pool(name="stat", bufs=1) as stat,
            tc.tile_pool(name="scr", bufs=1) as scr,
            tc.tile_pool(name="expb", bufs=4) as expb,
            tc.tile_pool(name="ybuf", bufs=3) as ybuf,
            tc.tile_pool(name="dsum", bufs=2) as dsum,
            tc.tile_pool(name="psum", bufs=2, space="PSUM") as psum,
        ):
            # ---- prologue: warm the exp ACT table + PE pstate during DMAs ----
            wu_a = scr.tile([128, KT, 128], _F8)
            wu_b = scr.tile([128, KT, 512], _F8)
            wu_e = scr.tile([128, 1], _F32)
            nc.gpsimd.memset(wu_a, 0.0)
            nc.gpsimd.memset(wu_b, 0.0)
            nc.gpsimd.memset(wu_e, 0.0)
            nc.scalar.activation(wu_e, wu_e, AF.Exp)  # pull exp table load early
            wu_p = psum.tile([128, 2048], _F32, tag="pt")
            for i in range(N_WARMUP_MM):
                nc.tensor.matmul(
                    wu_p[:, (i % 4) * 512 : (i % 4) * 512 + 512],
                    wu_a,
                    wu_b,
                    start=True,
                    stop=True,
                    perf_mode=DR,
                )

            # ---- input DMAs, ordered by when they gate compute ----
            xf_sb = big.tile([128, NT, D], _BF16)
            g_sb = big.tile([128, NT, D], _BF16)
            wt_sb = big.tile([128, KT, C], _F8)
            xt_sb = big.tile([128, KT, NS], _F8)

            def _wt_chunk(gi):
                c0 = sum(CG[:gi])
                w = CG[gi]
                for k in range(KT):
                    nc.sync.dma_start(
                        out=wt_sb[:, k, c0 : c0 + w], in_=wt_ext[:, k, c0 : c0 + w]
                    )

            _wt_chunk(0)
            for k in range(KT):
                nc.sync.dma_start(out=xt_sb[:, k, :], in_=xt_ext[:, k, :])
            _wt_chunk(1)
            nc.sync.dma_start(out=xf_sb[:, :NH, :], in_=xf_ext[:, :NH, :])
            _wt_chunk(2)
            _wt_chunk(3)
            nc.sync.dma_start(out=g_sb[:, :NH, :], in_=g_ext[:, :NH, :])
            _wt_chunk(4)
            nc.sync.dma_start(out=xf_sb[:, NH:, :], in_=xf_ext[:, NH:, :])
            nc.sync.dma_start(out=g_sb[:, NH:, :], in_=g_ext[:, NH:, :])

            # ---- per-(tile, group) partial exp-sums ----
            # separate per-half tiles so phase-3's reads of half 0 don't
            # create write-after-read deps against later esum writes
            esum_a = stat.tile([128, NH * NG], _F32)
            esum_b = stat.tile([128, NH * NG], _F32)
            esum_h = [esum_a, esum_b]

            rawt = stat.tile([128, NT], _F32)   # S * tgt
            num = stat.tile([128, NT], _F32)    # S * (tgt - M)
            dotscr = scr.tile([128, D], _BF16)  # STT main-out scratch

            def _mm_chunk(t, gi):
                c0 = sum(CG[:gi])
                w = CG[gi]
                pt = psum.tile([128, 2048], _F32, tag="pt")
                for b0 in range(0, w, 512):
                    bw = min(512, w - b0)
                    nc.tensor.matmul(
                        pt[:, b0 : b0 + bw],
                        xt_sb[:, :, t * 128 : (t + 1) * 128],
                        wt_sb[:, :, c0 + b0 : c0 + b0 + bw],
                        start=True,
                        stop=True,
                        perf_mode=DR,
                    )
                return pt, w

            def _slot(t, gi):
                h, th = (0, t) if t < NH else (1, t - NH)
                return esum_h[h], th * NG + gi

            def _act_chunk(t, gi):
                pt, w = _mm_chunk(t, gi)
                esum, idx = _slot(t, gi)
                eo = expb.tile([128, 2048], _BF16, tag="eo")
                nc.scalar.activation(
                    eo[:, :w],
                    pt[:, :w],
                    AF.Exp,
                    scale=1.0 / WSCALE,
                    accum_out=esum[:, idx : idx + 1],
                )

            def _dve_chunk(t, gi):
                pt, w = _mm_chunk(t, gi)
                esum, idx = _slot(t, gi)
                y = ybuf.tile([128, 2048], _I16, tag="y")
                # pass 1: i16 = rne(A16 * z16 + B16); bitcast(i16) ~ exp(z)
                nc.vector.tensor_scalar(
                    y[:, :w], pt[:, :w], A16, B16, mult, addop
                )
                yb = y.bitcast(_BF16)
                h2 = w // 2
                ds = dsum.tile([128, 1024], _BF16, tag="ds")
                # pass 2: halves-add + accumulate the fp32 row sum
                nc.vector.scalar_tensor_tensor(
                    out=ds[:, :h2],
                    in0=yb[:, :h2],
                    scalar=1.0,
                    in1=yb[:, h2:w],
                    op0=mult,
                    op1=addop,
                    accum_out=esum[:, idx : idx + 1],
                )

            def _dots(lo, hi):
                for t in range(lo, hi):
                    nc.vector.scalar_tensor_tensor(
                        out=dotscr,
                        in0=xf_sb[:, t, :],
                        scalar=1.0,
                        in1=g_sb[:, t, :],
                        op0=mult,
                        op1=mult,
                        accum_out=rawt[:, t : t + 1],
                    )

            # ---- phase-3 machinery (runs per half so half 0 hides under
            # the stream and only half 1 is on the tail) ----
            esum_vh = [e.rearrange("p (t g) -> p t g", g=NG) for e in esum_h]
            expn = stat.tile([128, NT], _F32)
            expt = stat.tile([128, NT], _F32)
            et = stat.tile([128, NT], _F32)
            denom = stat.tile([128, NT], _F32)
            ef = stat.tile([128, NT], _F32)
            mm = stat.tile([128, NT], _F32)
            acc = stat.tile([128, NT], _F32)
            L = stat.tile([128, NT], _F32)
            lsr = mybir.AluOpType.logical_shift_right
            band = mybir.AluOpType.bitwise_and
            bor = mybir.AluOpType.bitwise_or
            # ln(m) via degree-3 poly (max abs err 1.3e-3)
            PC = [
                1.0689890822e-01, -7.1197693854e-01, 2.0805856522e+00,
                -1.4741810531e+00,
            ]

            def _phase3(h):
                lo, hi = (0, NH) if h == 0 else (NH, NT)
                s = slice(lo, hi)
                nc.vector.reduce_sum(
                    et[:, s], esum_vh[h][:, :, :], axis=mybir.AxisListType.X
                )
                nc.vector.tensor_add(denom[:, s], et[:, s], expn[:, s])
                nc.vector.tensor_sub(denom[:, s], denom[:, s], expt[:, s])
                # ln(d) = ln2*e + p3(m), d = m * 2^e, m in [1,2)
                nc.vector.tensor_scalar(
                    acc[:, s].bitcast(_I32), denom[:, s].bitcast(_I32),
                    23, None, lsr,
                )
                nc.vector.tensor_scalar(
                    acc[:, s].bitcast(_I32), acc[:, s].bitcast(_I32),
                    127, None, sub,
                )
                nc.vector.tensor_copy(ef[:, s], acc[:, s].bitcast(_I32))
                nc.vector.tensor_scalar(
                    mm[:, s].bitcast(_I32), denom[:, s].bitcast(_I32),
                    0x7FFFFF, 0x3F800000, band, bor,
                )
                nc.vector.tensor_scalar(
                    acc[:, s], mm[:, s], PC[0], PC[1], mult, addop
                )
                nc.vector.tensor_mul(acc[:, s], acc[:, s], mm[:, s])
                nc.vector.tensor_scalar_add(acc[:, s], acc[:, s], PC[2])
                nc.vector.tensor_mul(acc[:, s], acc[:, s], mm[:, s])
                nc.vector.tensor_scalar_add(acc[:, s], acc[:, s], PC[3])
                nc.vector.scalar_tensor_tensor(
                    out=acc[:, s], in0=ef[:, s], scalar=LN2, in1=acc[:, s],
                    op0=mult, op1=addop,
                )
                nc.vector.tensor_sub(L[:, s], num[:, s], acc[:, s])
                nc.sync.dma_start(out=out_ext[:, s], in_=L[:, s])

            # ---- main stream: program order = per-engine schedule order ----
            for t in range(NT):
                dve_gis = _dve_gis(t)
                order = [0, 1, 2, 3, 4] if 1 in dve_gis else [0, 1, 2, 3, 4]
                for gi in order:
                    if gi in dve_gis:
                        _dve_chunk(t, gi)
                    else:
                        _act_chunk(t, gi)
                if t == 1:
                    _dots(0, NH)
                    nc.vector.tensor_scalar_add(
                        num[:, :NH], rawt[:, :NH], -SM
                    )
                if t == 3:
                    nc.scalar.activation(
                        expn[:, :NH], num[:, :NH], AF.Exp
                    )
                    nc.scalar.activation(
                        expt[:, :NH], rawt[:, :NH], AF.Exp
                    )
                if t == 8:
                    _dots(NH, NT)
                    nc.vector.tensor_scalar_add(
                        num[:, NH:], rawt[:, NH:], -SM
                    )
                if t == 10:
                    _phase3(0)
                if t == 12:
                    nc.scalar.activation(
                        expn[:, NH:], num[:, NH:], AF.Exp
                    )
                    nc.scalar.activation(
                        expt[:, NH:], rawt[:, NH:], AF.Exp
                    )
            _phase3(1)

    nc.finalize()
    return nc


_NC_CACHE = None


def _get_nc():
    global _NC_CACHE
    if _NC_CACHE is None:
        _NC_CACHE = _build_nc()
    return _NC_CACHE


def _shuffle_pm(a, nt):
    """[nt*128, d] row-major -> [128, nt, d] partition-major."""
    d = a.shape[-1]
    return np.ascontiguousarray(a.reshape(nt, 128, d).transpose(1, 0, 2))


def prep_core(xs, ls, W, wt=None):
    """Build one core's input map from its (pre-scaled) row block."""
    nt = xs.shape[0] // 128
    if wt is None:
        wt = _shuffle_pm(
            np.ascontiguousarray((WSCALE * W).T), KT
        ).astype(ml_dtypes.float8_e4m3)
    xt = _shuffle_pm(np.ascontiguousarray(xs.T), KT).astype(ml_dtypes.float8_e4m3)
    xf = _shuffle_pm(xs, nt).astype(ml_dtypes.bfloat16)
    g = _shuffle_pm(W[ls], nt).astype(ml_dtypes.bfloat16)
    return {"xt": xt, "wt": wt, "xf": xf, "g": g}


def make_in_maps(x, labels, W):
    x = np.asarray(x, dtype=np.float32)
    W = np.asarray(W, dtype=np.float32)
    labels = np.asarray(labels)
    # fold S / ||x_i|| into the embeddings on the host
    xs = x * (S / np.linalg.norm(x, axis=1, keepdims=True))
    wt = _shuffle_pm(
        np.ascontiguousarray((WSCALE * W).T), KT
    ).astype(ml_dtypes.float8_e4m3)
    return [
        prep_core(
            xs[i * NS : (i + 1) * NS], labels[i * NS : (i + 1) * NS], W, wt
        )
        for i in range(NCORES)
    ]


def run_device(x, labels, W, **kwargs):
    nc = _get_nc()
    in_maps = make_in_maps(x, labels, W)
    res = run_bass_kernel_spmd(nc, in_maps, list(range(NCORES)), **kwargs)
    return res


def finish(res):
    parts = []
    for i in range(NCORES):
        o = res.results[i]["out"]            # [128, NT]; row = t*128 + p
        parts.append(np.asarray(o).T.reshape(-1))
    L = np.concatenate(parts)
    return np.asarray(-np.mean(L), dtype=np.float32)


def kernel(x, labels, W):
    return finish(run_device(x, labels, W))


# revision 6
# speedup vs baseline: 1.1814x; 1.1814x over previous
"""AdMSoftmaxLoss fused distributed kernel for 8 TRN2 NeuronCores (v2).

Math (reference):
    xn = x / ||x||                     # row-L2-normalized embeddings
    wf = xn @ W.T                      # [N, C] logits
    tgt = wf[i, y_i]
    num = S * (tgt - M)
    excl = sum_c exp(S*wf) - exp(S*tgt)
    L = num - log(exp(num) + excl);  loss = -mean(L)

Strategy (v2): pure data-parallel over N (2048 rows/core), no collectives.
The scale S/||x_i|| is folded into x on the HOST (xs = S*x/||x||), so the
device matmul produces final logits directly and needs no per-row scale.
  - PE: fp8e4 DoubleRow matmuls (K=256 in one pass, 0.5 cyc/row): W is
    pre-scaled by 16 on host for fp8 range; the 1/16 is folded into the
    consumers.  PE stream ~35us -- far off the critical path.
  - The exp+row-sum work (20.5M elements/core) is SPLIT between the two
    engines that can read PSUM:
      * ScalarE (ACT): exp activation with scale=1/16 and accum_out
        (~2.28us per 2048-col chunk incl. the accumulator read), chunks
        {0,2,4} of each tile;
      * VectorE (DVE): Schraudolph bit-trick exp -- tensor_scalar affine
        fp32(PSUM) -> int16 bf16-bits (round-to-nearest), then one
        scalar_tensor_tensor that adds the two bf16 halves elementwise and
        accum-sums the result (~3.3us per chunk), chunks {1,3}.
    Schraudolph bf16 has ~+-4% sawtooth error, bias-corrected via the magic
    constant; per-row sums average it to ~0.3%, negligible vs the 2e-2 gate.
  - Target logits S*tgt come from DVE dot products of bf16 xs rows with the
    host-gathered g = W[labels] rows (accum_out).
  - Final log via the DVE exponent/poly bit-trick (no ACT table reload).
Per-row L values are DMA'd out; the host concatenates and means.

Measured: see test.py (baseline v1 was 189us HW; v2 targets ~120-130us).
"""

import numpy as np
import ml_dtypes

import concourse.mybir as mybir
import concourse.tile as tile
from concourse import bacc
from concourse.bass_utils import run_bass_kernel_spmd

N, D, C = 16384, 256, 10000
S, M = 30.0, 0.4
NCORES = 8
NS = N // NCORES      # 2048 rows per core
NT = NS // 128        # 16 n-tiles of 128 rows
KT = D // 128         # 2 k-slices (one DoubleRow pass)
CG = [2048, 2048, 2048, 2048, 1808]   # class-dim groups (sum = C)
NG = len(CG)
assert sum(CG) == C

_F32 = mybir.dt.float32
_BF16 = mybir.dt.bfloat16
_I16 = mybir.dt.int16
_I32 = mybir.dt.int32
_F8 = mybir.dt.float8e4

LN2 = float(np.log(2.0))
WSCALE = 16.0                       # host pre-scale on W for fp8 range
A16 = 128.0 / LN2 / WSCALE          # Schraudolph slope on 16x logits
B16 = 16256.0 - 7.37                # bf16 magic offset, mean-unbiased
SM = S * M

N_WARMUP_MM = 10


def _dve_gis(t):
    """Which class-groups of tile t the DVE (vs ACT) consumes."""
    return (2,) if t % 4 == 3 else (1, 3)


def _build_nc(ns=NS, cg=tuple(CG), c=C):
    nt = ns // 128
    cg = list(cg)
    nc = bacc.Bacc("TRN2", target_bir_lowering=False)
    AF = mybir.ActivationFunctionType
    NT, C = nt, c  # noqa: N806
    CG, NS = cg, ns  # noqa: N806
    NG = len(CG)  # noqa: N806
    NH = NT // 2  # noqa: N806
    DR = mybir.MatmulPerfMode.DoubleRow  # noqa: N806
    mult = mybir.AluOpType.mult
    sub = mybir.AluOpType.subtract
    addop = mybir.AluOpType.add

    xt_ext = nc.declare_dram_parameter("xt", [128, KT, NS], _F8, isOutput=False)
    wt_ext = nc.declare_dram_parameter("wt", [128, KT, C], _F8, isOutput=False)
    xf_ext = nc.declare_dram_parameter("xf", [128, NT, D], _BF16, isOutput=False)
    g_ext = nc.declare_dram_parameter("g", [128, NT, D], _BF16, isOutput=False)
    out_ext = nc.declare_dram_parameter("out", [128, NT], _F32, isOutput=True)

    with tile.TileContext(nc) as tc:
        with (
            tc.tile_pool(name="big", bufs=1) as big,
            tc.tile_pool(name="stat", bufs=1) as stat,
            tc.tile_pool(name="scr", bufs=1) as scr,
            tc.tile_pool(name="expb", bufs=4) as expb,
            tc.tile_pool(name="ybuf", bufs=3) as ybuf,
            tc.tile_pool(name="dsum", bufs=2) as dsum,
            tc.tile_pool(name="psum", bufs=2, space="PSUM") as psum,
        ):
            # ---- prologue: warm the exp ACT table + PE pstate during DMAs ----
            wu_a = scr.tile([128, KT, 128], _F8)
            wu_b = scr.tile([128, KT, 512], _F8)
            wu_e = scr.tile([128, 1], _F32)
            nc.gpsimd.memset(wu_a, 0.0)
            nc.gpsimd.memset(wu_b, 0.0)
            nc.gpsimd.memset(wu_e, 0.0)
            nc.scalar.activation(wu_e, wu_e, AF.Exp)  # pull exp table load early
            wu_p = psum.tile([128, 2048], _F32, tag="pt")
            for i in range(N_WARMUP_MM):
                nc.tensor.matmul(
                    wu_p[:, (i % 4) * 512 : (i % 4) * 512 + 512],
                    wu_a,
                    wu_b,
                    start=True,
                    stop=True,
                    perf_mode=DR,
                )

            # ---- input DMAs, ordered by when they gate compute ----
            xf_sb = big.tile([128, NT, D], _BF16)
            g_sb = big.tile([128, NT, D], _BF16)
            wt_sb = big.tile([128, KT, C], _F8)
            xt_sb = big.tile([128, KT, NS], _F8)

            def _wt_chunk(gi):
                c0 = sum(CG[:gi])
                w = CG[gi]
                for k in range(KT):
                    nc.sync.dma_start(
                        out=wt_sb[:, k, c0 : c0 + w], in_=wt_ext[:, k, c0 : c0 + w]
                    )

            _wt_chunk(0)
            for k in range(KT):
                nc.sync.dma_start(out=xt_sb[:, k, :], in_=xt_ext[:, k, :])
            _wt_chunk(1)
            nc.sync.dma_start(out=xf_sb[:, :NH, :], in_=xf_ext[:, :NH, :])
            _wt_chunk(2)
            _wt_chunk(3)
            nc.sync.dma_start(out=g_sb[:, :NH, :], in_=g_ext[:, :NH, :])
            _wt_chunk(4)
            nc.sync.dma_start(out=xf_sb[:, NH:, :], in_=xf_ext[:, NH:, :])
            nc.sync.dma_start(out=g_sb[:, NH:, :], in_=g_ext[:, NH:, :])

            # ---- per-(tile, group) partial exp-sums ----
            # separate per-half tiles so phase-3's reads of half 0 don't
            # create write-after-read deps against later esum writes
            esum_a = stat.tile([128, NH * NG], _F32)
            esum_b = stat.tile([128, NH * NG], _F32)
            esum_h = [esum_a, esum_b]

            rawt = stat.tile([128, NT], _F32)   # S * tgt
            num = stat.tile([128, NT], _F32)    # S * (tgt - M)
            dotscr = scr.tile([128, D], _BF16)  # STT main-out scratch

            def _mm_chunk(t, gi):
                c0 = sum(CG[:gi])
                w = CG[gi]
                pt = psum.tile([128, 2048], _F32, tag="pt")
                for b0 in range(0, w, 512):
                    bw = min(512, w - b0)
                    nc.tensor.matmul(
                        pt[:, b0 : b0 + bw],
                        xt_sb[:, :, t * 128 : (t + 1) * 128],
                        wt_sb[:, :, c0 + b0 : c0 + b0 + bw],
                        start=True,
                        stop=True,
                        perf_mode=DR,
                    )
                return pt, w

            def _slot(t, gi):
                h, th = (0, t) if t < NH else (1, t - NH)
                return esum_h[h], th * NG + gi

            def _act_chunk(t, gi):
                pt, w = _mm_chunk(t, gi)
                esum, idx = _slot(t, gi)
                eo = expb.tile([128, 2048], _BF16, tag="eo")
                nc.scalar.activation(
                    eo[:, :w],
                    pt[:, :w],
                    AF.Exp,
                    scale=1.0 / WSCALE,
                    accum_out=esum[:, idx : idx + 1],
                )

            def _dve_chunk(t, gi):
                pt, w = _mm_chunk(t, gi)
                esum, idx = _slot(t, gi)
                y = ybuf.tile([128, 2048], _I16, tag="y")
                # pass 1: i16 = rne(A16 * z16 + B16); bitcast(i16) ~ exp(z)
                nc.vector.tensor_scalar(
                    y[:, :w], pt[:, :w], A16, B16, mult, addop
                )
                yb = y.bitcast(_BF16)
                h2 = w // 2
                ds = dsum.tile([128, 1024], _BF16, tag="ds")
                # pass 2: halves-add + accumulate the fp32 row sum
                nc.vector.scalar_tensor_tensor(
                    out=ds[:, :h2],
                    in0=yb[:, :h2],
                    scalar=1.0,
                    in1=yb[:, h2:w],
                    op0=mult,
                    op1=addop,
                    accum_out=esum[:, idx : idx + 1],
                )

            def _dots(lo, hi):
                for t in range(lo, hi):
                    nc.vector.scalar_tensor_tensor(
                        out=dotscr,
                        in0=xf_sb[:, t, :],
                        scalar=1.0,
                        in1=g_sb[:, t, :],
                        op0=mult,
                        op1=mult,
                        accum_out=rawt[:, t : t + 1],
                    )

            # ---- phase-3 machinery (runs per half so half 0 hides under
            # the stream and only half 1 is on the tail) ----
            esum_vh = [e.rearrange("p (t g) -> p t g", g=NG) for e in esum_h]
            expn = stat.tile([128, NT], _F32)
            expt = stat.tile([128, NT], _F32)
            et = stat.tile([128, NT], _F32)
            denom = stat.tile([128, NT], _F32)
            ef = stat.tile([128, NT], _F32)
            mm = stat.tile([128, NT], _F32)
            acc = stat.tile([128, NT], _F32)
            L = stat.tile([128, NT], _F32)
            lsr = mybir.AluOpType.logical_shift_right
            band = mybir.AluOpType.bitwise_and
            bor = mybir.AluOpType.bitwise_or
            # ln(m) via degree-3 poly (max abs err 1.3e-3)
            PC = [
                1.0689890822e-01, -7.1197693854e-01, 2.0805856522e+00,
                -1.4741810531e+00,
            ]

            def _phase3(h):
                lo, hi = (0, NH) if h == 0 else (NH, NT)
                s = slice(lo, hi)
                nc.vector.reduce_sum(
                    et[:, s], esum_vh[h][:, :, :], axis=mybir.AxisListType.X
                )
                nc.vector.tensor_add(denom[:, s], et[:, s], expn[:, s])
                nc.vector.tensor_sub(denom[:, s], denom[:, s], expt[:, s])
                # ln(d) = ln2*e + p3(m), d = m * 2^e, m in [1,2)
                nc.vector.tensor_scalar(
                    acc[:, s].bitcast(_I32), denom[:, s].bitcast(_I32),
                    23, None, lsr,
                )
                nc.vector.tensor_scalar(
                    acc[:, s].bitcast(_I32), acc[:, s].bitcast(_I32),
                    127, None, sub,
                )
                nc.vector.tensor_copy(ef[:, s], acc[:, s].bitcast(_I32))
                nc.vector.tensor_scalar(
                    mm[:, s].bitcast(_I32), denom[:, s].bitcast(_I32),
                    0x7FFFFF, 0x3F800000, band, bor,
                )
                nc.vector.tensor_scalar(
                    acc[:, s], mm[:, s], PC[0], PC[1], mult, addop
                )
                nc.vector.tensor_mul(acc[:, s], acc[:, s], mm[:, s])
                nc.vector.tensor_scalar_add(acc[:, s], acc[:, s], PC[2])
                nc.vector.tensor_mul(acc[:, s], acc[:, s], mm[:, s])
                nc.vector.tensor_scalar_add(acc[:, s], acc[:, s], PC[3])
                nc.vector.scalar_tensor_tensor(
                    out=acc[:, s], in0=ef[:, s], scalar=LN2, in1=acc[:, s],
                    op0=mult, op1=addop,
                )
                nc.vector.tensor_sub(L[:, s], num[:, s], acc[:, s])
                nc.sync.dma_start(out=out_ext[:, s], in_=L[:, s])

            # ---- main stream: program order = per-engine schedule order ----
            for t in range(NT):
                dve_gis = _dve_gis(t)
                order = [0, 1, 2, 3, 4] if 1 in dve_gis else [0, 1, 2, 3, 4]
                for gi in order:
                    if gi in dve_gis:
                        _dve_chunk(t, gi)
                    else:
                        _act_chunk(t, gi)
                if t == 1:
                    _dots(0, NH)
                    nc.vector.tensor_scalar_add(
                        num[:, :NH], rawt[:, :NH], -SM
                    )
                if t == 3:
                    nc.scalar.activation(
                        expn[:, :NH], num[:, :NH], AF.Exp
                    )
                    nc.scalar.activation(
                        expt[:, :NH], rawt[:, :NH], AF.Exp
                    )
                if t == 8:
                    _dots(NH, NT)
                    nc.vector.tensor_scalar_add(
                        num[:, NH:], rawt[:, NH:], -SM
                    )
                if t == 10:
                    _phase3(0)
                if t == 12:
                    nc.scalar.activation(
                        expn[:, NH:], num[:, NH:], AF.Exp
                    )
                    nc.scalar.activation(
                        expt[:, NH:], rawt[:, NH:], AF.Exp
                    )
            _phase3(1)

    nc.finalize()
    return nc


_NC_CACHE = None


def _get_nc():
    global _NC_CACHE
    if _NC_CACHE is None:
        _NC_CACHE = _build_nc()
    return _NC_CACHE


def _shuffle_pm(a, nt):
    """[nt*128, d] row-major -> [128, nt, d] partition-major."""
    d = a.shape[-1]
    return np.ascontiguousarray(a.reshape(nt, 128, d).transpose(1, 0, 2))


def prep_core(xs, ls, W, wt=None):
    """Build one core's input map from its (pre-scaled) row block."""
    nt = xs.shape[0] // 128
    if wt is None:
        wt = _shuffle_pm(
            np.ascontiguousarray((WSCALE * W).T), KT
        ).astype(ml_dtypes.float8_e4m3)
    xt = _shuffle_pm(np.ascontiguousarray(xs.T), KT).astype(ml_dtypes.float8_e4m3)
    xf = _shuffle_pm(xs, nt).astype(ml_dtypes.bfloat16)
    g = _shuffle_pm(W[ls], nt).astype(ml_dtypes.bfloat16)
    return {"xt": xt, "wt": wt, "xf": xf, "g": g}


def make_in_maps(x, labels, W):
    x = np.asarray(x, dtype=np.float32)
    W = np.asarray(W, dtype=np.float32)
    labels = np.asarray(labels)
    # fold S / ||x_i|| into the embeddings on the host
    xs = x * (S / np.linalg.norm(x, axis=1, keepdims=True))
    wt = _shuffle_pm(
        np.ascontiguousarray((WSCALE * W).T), KT
    ).astype(ml_dtypes.float8_e4m3)
    return [
        prep_core(
            xs[i * NS : (i + 1) * NS], labels[i * NS : (i + 1) * NS], W, wt
        )
        for i in range(NCORES)
    ]


def run_device(x, labels, W, **kwargs):
    nc = _get_nc()
    in_maps = make_in_maps(x, labels, W)
    res = run_bass_kernel_spmd(nc, in_maps, list(range(NCORES)), **kwargs)
    return res


def finish(res):
    parts = []
    for i in range(NCORES):
        o = res.results[i]["out"]            # [128, NT]; row = t*128 + p
        parts.append(np.asarray(o).T.reshape(-1))
    L = np.concatenate(parts)
    return np.asarray(-np.mean(L), dtype=np.float32)


def kernel(x, labels, W):
    return finish(run_device(x, labels, W))
